# revision 4
# baseline (speedup 1.0000x reference)
"""Expert-parallel MoE (top-2 of 8, BitNet QAT) kernel for 8 Trainium2 NeuronCores.

Strategy (one expert per core, SPMD — every core runs the same program):
  - x replicated (plus a zero pad row); batch dim of x data-parallel sharded for
    the router; w_in/w_out sharded along the expert axis; w_gate replicated.
  - On-device fp32 router (PE) + top-2 (DVE Max8) + gates + aux-loss partials.
  - AllGather of a tiny per-expert gate payload; on-device compaction of each
    expert's token list (prefix sums + one indirect scatter).
  - Indirect gather of routed token rows; activations quantized to INTEGER
    bf16 levels so the PE GEMMs are exact; Silu GLU on ScalarE; re-quantize;
    second GEMM; per-token gate/dequant scaling; compacted rows out.
  - Host: index-add combine of per-expert rows + final loss reduction.
"""
import contextlib
import ctypes
import os
import sys
import types

import numpy as np

os.environ.setdefault("JAX_PLATFORMS", "axon")

# ---------------------------------------------------------------------------
# antenv.axon_hooks shim: run_bass_kernel_spmd(trace=True) (e.g. under
# BASS_TRACE=1) imports this module; provide the standard ctypes NTFF hook if
# the image lacks it so tracing degrades gracefully instead of crashing.
_SO_PATH = "/opt/axon/libaxon_pjrt.so"


def _make_ntff_hook():
    if not os.path.exists(_SO_PATH):
        return None
    lib = ctypes.CDLL(_SO_PATH)
    if not hasattr(lib, "axon_start_nrt_profile"):
        return None
    lib.axon_start_nrt_profile.argtypes = [ctypes.POINTER(ctypes.c_int64), ctypes.c_size_t]
    lib.axon_start_nrt_profile.restype = ctypes.c_int64
    lib.axon_stop_nrt_profile.argtypes = [ctypes.c_char_p]
    lib.axon_stop_nrt_profile.restype = ctypes.c_int64

    @contextlib.contextmanager
    def _hook(output_dir, device_ids=None):
        import jax

        jax.devices()
        if device_ids:
            ids = (ctypes.c_int64 * len(device_ids))(*device_ids)
            rc = lib.axon_start_nrt_profile(ids, len(device_ids))
        else:
            rc = lib.axon_start_nrt_profile(None, 0)
        if rc != 0:
            raise RuntimeError(f"axon_start_nrt_profile rc={rc}")
        try:
            yield
        finally:
            lib.axon_stop_nrt_profile(output_dir.encode())

    return _hook


def _install_hooks():
    if "antenv.axon_hooks" in sys.modules:
        return
    hook = [_make_ntff_hook()]
    mod = types.ModuleType("antenv.axon_hooks")
    mod.get_axon_ntff_profile_hook = lambda: hook[0]

    def _set(h):
        hook[0] = h

    mod.set_axon_ntff_profile_hook = _set
    sys.modules["antenv.axon_hooks"] = mod
    try:
        import antenv

        antenv.axon_hooks = mod
    except ImportError:
        pass


_install_hooks()

import concourse.bacc as bacc
import concourse.mybir as mybir
import concourse.tile as tile
from concourse.bass import IndirectOffsetOnAxis
from concourse.bass_interp import get_hw_module
from concourse.bass_utils import run_bass_kernel_spmd
from concourse.masks import make_identity

F32 = mybir.dt.float32
BF16 = mybir.dt.bfloat16
I32 = mybir.dt.int32
U32 = mybir.dt.uint32
AF = mybir.ActivationFunctionType
OP = mybir.AluOpType
AX = mybir.AxisListType
MAGIC = float(np.float32(1.5 * 2**23))
P = 128


class Cfg:
    def __init__(self, D=1024, H=2048, E=8, NTOK=8192, CAP=2304, CORES=8):
        self.D, self.H, self.E, self.NTOK, self.CAP, self.CORES = D, H, E, NTOK, CAP, CORES
        self.H2 = 2 * H
        self.DCH = D // P
        self.HCH = H // P
        self.SH = NTOK // CORES
        self.SHT = self.SH // P
        self.NT = CAP // P
        self.NA = NTOK // P
        self.LGROWS = CAP + P
        self.G1W = min(512, self.H)
        self.NPAIR = H // self.G1W


def build(cfg):
    c = cfg
    nc = bacc.Bacc("TRN2", target_bir_lowering=False, debug=False,
                   num_devices=c.CORES, enable_asserts=False)
    xpad = nc.dram_tensor("xpad", [c.NTOK + 1, c.D], F32, kind="ExternalInput").ap()
    xsh = nc.dram_tensor("xsh", [c.SH, c.D], F32, kind="ExternalInput").ap()
    wg = nc.dram_tensor("wg", [c.D, c.E], F32, kind="ExternalInput").ap()
    w1 = nc.dram_tensor("w1", [c.H2, c.D], F32, kind="ExternalInput").ap()
    w2 = nc.dram_tensor("w2", [c.D, c.H], F32, kind="ExternalInput").ap()
    eidrows = nc.dram_tensor("eidrows", [8, 1], I32, kind="ExternalInput").ap()
    rows_out = nc.dram_tensor("rows_out", [c.CAP, c.D], F32, kind="ExternalOutput").ap()
    lg_out = nc.dram_tensor("lg_out", [c.LGROWS, 2], F32, kind="ExternalOutput").ap()
    lossp = nc.dram_tensor("lossp", [1, 32], F32, kind="ExternalOutput").ap()
    payload = nc.dram_tensor("payload", [8, c.SH], F32).ap()
    pay_all = nc.dram_tensor("pay_all", [8 * c.CORES, c.SH], F32, addr_space="Shared").ap()
    paybounce = nc.dram_tensor("paybounce", [c.NTOK], F32).ap()

    with tile.TileContext(nc, trace_sim=False) as tc:
        _build_body(tc, nc, c, xpad, xsh, wg, w1, w2, eidrows,
                    rows_out, lg_out, lossp, payload, pay_all, paybounce)
    nc.compile()
    nc.m = get_hw_module(nc.m)
    return nc


def _build_body(tc, nc, c, xpad, xsh, wg, w1, w2, eidrows,
                rows_out, lg_out, lossp, payload, pay_all, paybounce):
    ctx = contextlib.ExitStack()
    with ctx:
        cpool = ctx.enter_context(tc.tile_pool(name="consts", bufs=1))
        wtp = ctx.enter_context(tc.tile_pool(name="wt", bufs=1))
        stg = ctx.enter_context(tc.tile_pool(name="stage", bufs=2))
        rtp = ctx.enter_context(tc.tile_pool(name="router", bufs=2))
        acc = ctx.enter_context(tc.tile_pool(name="acc", bufs=1))
        cmp_ = ctx.enter_context(tc.tile_pool(name="compact", bufs=1))
        mn = ctx.enter_context(tc.tile_pool(name="main", bufs=2))
        mn1 = ctx.enter_context(tc.tile_pool(name="main1", bufs=1))
        sm = ctx.enter_context(tc.tile_pool(name="small", bufs=4))
        ps_s = ctx.enter_context(tc.tile_pool(name="ps_small", bufs=2, space="PSUM"))
        ps_g1 = ctx.enter_context(tc.tile_pool(name="ps_g1", bufs=4, space="PSUM"))
        ps_g2 = ctx.enter_context(tc.tile_pool(name="ps_g2", bufs=2, space="PSUM"))

        # constants
        idn_f = cpool.tile([P, P], F32)
        make_identity(nc, idn_f[:])
        idn_b = cpool.tile([P, P], BF16)
        make_identity(nc, idn_b[:])
        ones_col = cpool.tile([P, 1], F32)
        nc.vector.memset(ones_col[:], 1.0)
        ones_row = cpool.tile([1, P], F32)
        nc.vector.memset(ones_row[:], 1.0)
        strictL = cpool.tile([P, P], F32)
        nc.vector.memset(strictL[:], 1.0)
        nc.gpsimd.affine_select(
            out=strictL[:], in_=strictL[:], compare_op=OP.is_gt, fill=0.0,
            base=0, pattern=[[1, P]], channel_multiplier=-1)
        expiota = cpool.tile([P, c.E], I32)
        nc.gpsimd.iota(expiota[:], pattern=[[1, c.E]], base=0, channel_multiplier=0)
        expiota_f = cpool.tile([P, c.E], F32)
        nc.vector.tensor_copy(expiota_f[:], expiota[:])

        def bcast_scalar(src11, name):
            pt = ps_s.tile([P, 1], F32, tag="s")
            nc.tensor.matmul(pt[:], lhsT=ones_row[:], rhs=src11[:], start=True, stop=True)
            out = cpool.tile([P, 1], F32, tag=f"bc_{name}")
            nc.vector.tensor_copy(out[:], pt[:])
            return out

        # ---- weight quantization: fp32 [R, C] -> ternary bf16 transposed ----
        def quant_weights(wsrc, R, C, wt_sb, tag):
            RT = R // P
            colacc = acc.tile([P, RT], F32, tag=f"colacc_{tag}")
            for r in range(RT):
                wtile = stg.tile([P, C], F32, tag="wstage")
                nc.sync.dma_start(out=wtile[:], in_=wsrc[r * P:(r + 1) * P, :])
                nc.vector.tensor_reduce(colacc[:, r:r + 1], wtile[:], AX.X, OP.add,
                                        apply_absolute_value=True)
            rowsum = sm.tile([P, 1], F32, tag=f"ws_{tag}")
            nc.vector.tensor_reduce(rowsum[:], colacc[:], AX.X, OP.add)
            tot_ps = ps_s.tile([1, 1], F32, tag="s")
            nc.tensor.matmul(tot_ps[:], lhsT=ones_col[:], rhs=rowsum[:], start=True, stop=True)
            mean11 = sm.tile([1, 4], F32, tag=f"mean_{tag}")
            nc.vector.tensor_scalar(mean11[:, 0:1], tot_ps[:], float(1.0 / (R * C)), 1e-5,
                                    op0=OP.mult, op1=OP.max)
            nc.vector.reciprocal(mean11[:, 1:2], mean11[:, 0:1])
            nc.vector.reciprocal(mean11[:, 2:3], mean11[:, 1:2])
            s_b = bcast_scalar(mean11[:, 1:2], f"s_{tag}")
            cw_b = bcast_scalar(mean11[:, 2:3], f"cw_{tag}")
            for r in range(RT):
                wtile = stg.tile([P, C], F32, tag="wstage")
                nc.sync.dma_start(out=wtile[:], in_=wsrc[r * P:(r + 1) * P, :])
                nc.vector.tensor_scalar(wtile[:], wtile[:], s_b[:], MAGIC, op0=OP.mult, op1=OP.add)
                nc.vector.tensor_scalar(wtile[:], wtile[:], MAGIC, 1.0, op0=OP.subtract, op1=OP.min)
                qb = stg.tile([P, C], BF16, tag="wqb")
                nc.vector.tensor_scalar(qb[:], wtile[:], -1.0, None, op0=OP.max)
                for cb in range(C // P):
                    pt = ps_s.tile([P, P], BF16, tag="s")
                    nc.tensor.transpose(pt[:], qb[:, cb * P:(cb + 1) * P], idn_b[:])
                    nc.scalar.copy(out=wt_sb[:, cb, r * P:(r + 1) * P], in_=pt[:])
            return cw_b

        w1t = wtp.tile([P, c.DCH, c.H2], BF16)
        w2t = wtp.tile([P, c.HCH, c.D], BF16)
        cw1_b = quant_weights(w1, c.H2, c.D, w1t, "w1")
        cw2_b = quant_weights(w2, c.D, c.H, w2t, "w2")

        # ---- router on this core's shard ----
        wg_sb = cpool.tile([P, c.DCH, c.E], F32)
        nc.sync.dma_start(out=wg_sb[:], in_=wg[:].rearrange("(k p) e -> p k e", p=P))
        p_acc = acc.tile([P, c.E], F32)
        f_acc = acc.tile([P, c.E], F32)
        z_acc = acc.tile([P, 1], F32)
        nc.vector.memset(p_acc[:], 0.0)
        nc.vector.memset(f_acc[:], 0.0)
        nc.vector.memset(z_acc[:], 0.0)
        payT = acc.tile([8, c.SH], F32)

        for t in range(c.SHT):
            xst = stg.tile([P, c.D], F32, tag="wstage")
            nc.sync.dma_start(out=xst[:], in_=xsh[t * P:(t + 1) * P, :])
            xT = rtp.tile([P, c.DCH, P], F32, tag="xT")
            for k in range(c.DCH):
                pt = ps_s.tile([P, P], F32, tag="s")
                nc.tensor.transpose(pt[:], xst[:, k * P:(k + 1) * P], idn_f[:])
                nc.scalar.copy(out=xT[:, k, :], in_=pt[:])
            lps = ps_s.tile([P, c.E], F32, tag="s")
            for k in range(c.DCH):
                nc.tensor.matmul(lps[:], lhsT=xT[:, k, :], rhs=wg_sb[:, k, :],
                                 start=(k == 0), stop=(k == c.DCH - 1))
            lt = rtp.tile([P, c.E], F32, tag="lt")
            nc.vector.tensor_copy(lt[:], lps[:])
            mx8 = rtp.tile([P, 8], F32, tag="mx8")
            nc.vector.max(out=mx8[:], in_=lt[:])
            ix8 = rtp.tile([P, 8], U32, tag="ix8")
            nc.vector.max_index(out=ix8[:], in_max=mx8[:], in_values=lt[:])
            ixf = rtp.tile([P, 2], F32, tag="ixf")
            nc.vector.tensor_copy(ixf[:], ix8[:, 0:2])
            v1 = mx8[:, 0:1]
            v2 = mx8[:, 1:2]
            gsc = rtp.tile([P, 6], F32, tag="gsc")
            nc.vector.tensor_tensor(out=gsc[:, 0:1], in0=v2, in1=v1, op=OP.subtract)
            nc.scalar.activation(gsc[:, 1:2], gsc[:, 0:1], AF.Exp)
            nc.vector.tensor_scalar_add(gsc[:, 2:3], gsc[:, 1:2], 1.0)
            nc.vector.reciprocal(gsc[:, 3:4], gsc[:, 2:3])
            nc.vector.tensor_tensor(out=gsc[:, 4:5], in0=gsc[:, 1:2], in1=gsc[:, 3:4], op=OP.mult)
            eq1 = rtp.tile([P, c.E], F32, tag="eq1")
            eq2 = rtp.tile([P, c.E], F32, tag="eq2")
            nc.vector.tensor_scalar(eq1[:], expiota_f[:], ixf[:, 0:1], None, op0=OP.is_equal)
            nc.vector.tensor_scalar(eq2[:], expiota_f[:], ixf[:, 1:2], None, op0=OP.is_equal)
            pay = rtp.tile([P, c.E], F32, tag="pay")
            nc.vector.tensor_scalar(pay[:], eq1[:], gsc[:, 3:4], None, op0=OP.mult)
            p2t = rtp.tile([P, c.E], F32, tag="p2t")
            nc.vector.tensor_scalar(p2t[:], eq2[:], gsc[:, 4:5], None, op0=OP.mult)
            nc.vector.tensor_add(pay[:], pay[:], p2t[:])
            sh_ = rtp.tile([P, c.E], F32, tag="sh")
            nc.vector.tensor_scalar(sh_[:], lt[:], v1, None, op0=OP.subtract)
            ex = rtp.tile([P, c.E], F32, tag="ex")
            ssum = rtp.tile([P, 1], F32, tag="ssum")
            nc.scalar.activation(ex[:], sh_[:], AF.Exp, accum_out=ssum[:])
            rs = rtp.tile([P, 1], F32, tag="rs")
            nc.vector.reciprocal(rs[:], ssum[:])
            probs = rtp.tile([P, c.E], F32, tag="probs")
            nc.vector.tensor_scalar(probs[:], ex[:], rs[:], None, op0=OP.mult)
            nc.vector.tensor_add(p_acc[:], p_acc[:], probs[:])
            fr = rtp.tile([P, c.E], F32, tag="fr")
            nc.vector.tensor_add(fr[:], eq1[:], eq2[:])
            nc.vector.tensor_add(f_acc[:], f_acc[:], fr[:])
            nc.scalar.activation(gsc[:, 5:6], ssum[:], AF.Ln)
            lse = rtp.tile([P, 1], F32, tag="lse")
            nc.vector.tensor_tensor(out=lse[:], in0=gsc[:, 5:6], in1=v1, op=OP.add)
            lse2 = rtp.tile([P, 1], F32, tag="lse2")
            nc.vector.tensor_tensor(out=lse2[:], in0=lse[:], in1=lse[:], op=OP.mult)
            nc.vector.tensor_add(z_acc[:], z_acc[:], lse2[:])
            ptp = ps_s.tile([c.E, P], F32, tag="s")
            nc.tensor.transpose(ptp[:], pay[:], idn_f[:])
            nc.scalar.copy(out=payT[0:c.E, t * P:(t + 1) * P], in_=ptp[:])

        lp_sb = acc.tile([1, 32], F32)
        nc.vector.memset(lp_sb[:], 0.0)
        red_ps = ps_s.tile([1, c.E], F32, tag="s")
        nc.tensor.matmul(red_ps[:], lhsT=ones_col[:], rhs=p_acc[:], start=True, stop=True)
        nc.vector.tensor_copy(lp_sb[:, 0:c.E], red_ps[:])
        red_ps2 = ps_s.tile([1, c.E], F32, tag="s")
        nc.tensor.matmul(red_ps2[:], lhsT=ones_col[:], rhs=f_acc[:], start=True, stop=True)
        nc.vector.tensor_copy(lp_sb[:, c.E:2 * c.E], red_ps2[:])
        red_ps3 = ps_s.tile([1, 1], F32, tag="s")
        nc.tensor.matmul(red_ps3[:], lhsT=ones_col[:], rhs=z_acc[:], start=True, stop=True)
        nc.vector.tensor_copy(lp_sb[:, 16:17], red_ps3[:])
        nc.sync.dma_start(out=lossp[:], in_=lp_sb[:])

        # ---- AllGather + relayout ----
        nc.sync.dma_start(out=payload[:], in_=payT[0:8, :])
        nc.gpsimd.collective_compute(
            "AllGather", OP.bypass, ins=[payload[:]], outs=[pay_all[:]],
            replica_groups=[list(range(c.CORES))])
        eid_sb = cmp_.tile([8, 1], I32)
        nc.sync.dma_start(out=eid_sb[:], in_=eidrows[:])
        pay_e = cmp_.tile([8, c.SH], F32)
        nc.gpsimd.indirect_dma_start(
            out=pay_e[:], out_offset=None, in_=pay_all[:],
            in_offset=IndirectOffsetOnAxis(ap=eid_sb[:, 0:1], axis=0))
        nc.sync.dma_start(out=paybounce[:], in_=pay_e[:])
        gate_col = cmp_.tile([P, c.NA], F32)
        nc.sync.dma_start(out=gate_col[:],
                          in_=paybounce[:].rearrange("(f p) -> p f", p=P))

        # ---- compaction ----
        mask = cmp_.tile([P, c.NA], F32)
        nc.vector.tensor_scalar(mask[:], gate_col[:], 0.0, None, op0=OP.is_gt)
        csa = cmp_.tile([P, c.NA], F32)
        csb = cmp_.tile([P, c.NA], F32)
        nc.vector.tensor_copy(csa[:], mask[:])
        a_cur, b_cur = csa, csb
        shift = 1
        while shift < c.NA:
            nc.vector.tensor_copy(b_cur[:, 0:shift], a_cur[:, 0:shift])
            nc.vector.tensor_add(b_cur[:, shift:c.NA], a_cur[:, shift:c.NA],
                                 a_cur[:, 0:c.NA - shift])
            a_cur, b_cur = b_cur, a_cur
            shift *= 2
        incl = a_cur
        off_ps = ps_s.tile([P, 1], F32, tag="s")
        nc.tensor.matmul(off_ps[:], lhsT=strictL[:], rhs=incl[:, c.NA - 1:c.NA],
                         start=True, stop=True)
        off = cmp_.tile([P, 1], F32)
        nc.vector.tensor_copy(off[:], off_ps[:])
        pos = cmp_.tile([P, c.NA], F32)
        nc.vector.tensor_sub(pos[:], incl[:], mask[:])
        nc.vector.tensor_scalar(pos[:], pos[:], off[:], None, op0=OP.add)
        mask_i = cmp_.tile([P, c.NA], I32)
        nc.vector.tensor_copy(mask_i[:], mask[:])
        slotf = cmp_.tile([P, c.NA], F32)
        nc.vector.memset(slotf[:], float(c.CAP))
        nc.vector.copy_predicated(slotf[:], mask_i[:], pos[:])
        slot_i = cmp_.tile([P, c.NA], I32)
        nc.vector.tensor_copy(slot_i[:], slotf[:])
        tid = cmp_.tile([P, c.NA], I32)
        nc.gpsimd.iota(tid[:], pattern=[[P, c.NA]], base=0, channel_multiplier=1)
        tidf = cmp_.tile([P, c.NA], F32)
        nc.vector.tensor_copy(tidf[:], tid[:])
        packed = cmp_.tile([P, c.NA, 2], F32)
        nc.vector.tensor_copy(packed[:, :, 0], tidf[:])
        nc.vector.tensor_copy(packed[:, :, 1], gate_col[:])
        lgw = (c.LGROWS * 2) // P
        lginit = cmp_.tile([P, lgw], F32)
        nc.vector.memset(lginit[:], 0.0)
        nc.vector.memset(lginit[:, 0:lgw:2], float(c.NTOK))
        nc.sync.dma_start(out=lg_out[:], in_=lginit[:])
        for f in range(c.NA):
            nc.gpsimd.indirect_dma_start(
                out=lg_out[:], out_offset=IndirectOffsetOnAxis(ap=slot_i[:, f:f + 1], axis=0),
                in_=packed[:, f, :], in_offset=None)
        tokf = cmp_.tile([P, c.NT], F32)
        nc.sync.dma_start(out=tokf[:],
                          in_=lg_out[0:c.CAP, 0:1].rearrange("(t p) one -> p (t one)", p=P))
        gates_s = cmp_.tile([P, c.NT], F32)
        nc.sync.dma_start(out=gates_s[:],
                          in_=lg_out[0:c.CAP, 1:2].rearrange("(t p) one -> p (t one)", p=P))
        tok_i = cmp_.tile([P, c.NT], I32)
        nc.vector.tensor_copy(tok_i[:], tokf[:])

        # ---- expert main loop ----
        for T in range(c.NT):
            xrow = mn.tile([P, c.D], F32, tag="xrow")
            nc.gpsimd.indirect_dma_start(
                out=xrow[:], out_offset=None, in_=xpad[:],
                in_offset=IndirectOffsetOnAxis(ap=tok_i[:, T:T + 1], axis=0))
            mx = sm.tile([P, 8], F32, tag="mx")
            nc.vector.tensor_reduce(mx[:, 0:1], xrow[:], AX.X, OP.max,
                                    apply_absolute_value=True)
            nc.vector.tensor_scalar(mx[:, 1:2], mx[:, 0:1], 1e-5, None, op0=OP.max)
            nc.vector.reciprocal(mx[:, 2:3], mx[:, 1:2])
            nc.vector.tensor_scalar(mx[:, 3:4], mx[:, 2:3], 127.0, None, op0=OP.mult)
            nc.vector.reciprocal(mx[:, 4:5], mx[:, 3:4])
            nc.vector.tensor_scalar(xrow[:], xrow[:], mx[:, 3:4], MAGIC, op0=OP.mult, op1=OP.add)
            xq = mn.tile([P, c.D], BF16, tag="xq")
            nc.vector.tensor_scalar(xq[:], xrow[:], MAGIC, None, op0=OP.subtract)
            xqT = mn.tile([P, c.DCH, P], BF16, tag="xqT")
            for k in range(c.DCH):
                pt = ps_s.tile([P, P], BF16, tag="s")
                nc.tensor.transpose(pt[:], xq[:, k * P:(k + 1) * P], idn_b[:])
                nc.scalar.copy(out=xqT[:, k, :], in_=pt[:])
            c1 = sm.tile([P, 1], F32, tag="c1")
            nc.vector.tensor_tensor(out=c1[:], in0=mx[:, 4:5], in1=cw1_b[:], op=OP.mult)
            u = mn1.tile([P, c.H], F32, tag="u")
            for j in range(c.NPAIR):
                psA = ps_g1.tile([P, c.G1W], F32, tag="g1")
                psB = ps_g1.tile([P, c.G1W], F32, tag="g1")
                for k in range(c.DCH):
                    nc.tensor.matmul(psA[:], lhsT=xqT[:, k, :],
                                     rhs=w1t[:, k, j * c.G1W:(j + 1) * c.G1W],
                                     start=(k == 0), stop=(k == c.DCH - 1))
                for k in range(c.DCH):
                    nc.tensor.matmul(psB[:], lhsT=xqT[:, k, :],
                                     rhs=w1t[:, k, c.H + j * c.G1W:c.H + (j + 1) * c.G1W],
                                     start=(k == 0), stop=(k == c.DCH - 1))
                sil = mn.tile([P, c.G1W], F32, tag="sil")
                nc.scalar.activation(sil[:], psA[:], AF.Silu, scale=c1[:])
                nc.vector.tensor_tensor(out=u[:, j * c.G1W:(j + 1) * c.G1W],
                                        in0=sil[:], in1=psB[:], op=OP.mult)
            um = sm.tile([P, 8], F32, tag="um")
            nc.vector.tensor_reduce(um[:, 0:1], u[:], AX.X, OP.max,
                                    apply_absolute_value=True)
            nc.vector.tensor_tensor(out=um[:, 1:2], in0=um[:, 0:1], in1=c1[:], op=OP.mult)
            nc.vector.tensor_scalar(um[:, 2:3], um[:, 1:2], 1e-5, None, op0=OP.max)
            nc.vector.reciprocal(um[:, 3:4], um[:, 2:3])
            nc.vector.tensor_scalar(um[:, 4:5], um[:, 3:4], 127.0, None, op0=OP.mult)
            nc.vector.reciprocal(um[:, 5:6], um[:, 4:5])
            cs = sm.tile([P, 1], F32, tag="cs")
            nc.vector.tensor_tensor(out=cs[:], in0=c1[:], in1=um[:, 4:5], op=OP.mult)
            nc.vector.tensor_scalar(u[:], u[:], cs[:], MAGIC, op0=OP.mult, op1=OP.add)
            m_bf = mn.tile([P, c.H], BF16, tag="m_bf")
            nc.vector.tensor_scalar(m_bf[:], u[:], MAGIC, None, op0=OP.subtract)
            mT = mn.tile([P, c.HCH, P], BF16, tag="mT")
            for hc in range(c.HCH):
                pt = ps_s.tile([P, P], BF16, tag="s")
                nc.tensor.transpose(pt[:], m_bf[:, hc * P:(hc + 1) * P], idn_b[:])
                nc.scalar.copy(out=mT[:, hc, :], in_=pt[:])
            fs = sm.tile([P, 1], F32, tag="fs")
            nc.vector.tensor_tensor(out=fs[:], in0=um[:, 5:6], in1=cw2_b[:], op=OP.mult)
            nc.vector.tensor_tensor(out=fs[:], in0=fs[:], in1=gates_s[:, T:T + 1], op=OP.mult)
            yout = mn.tile([P, c.D], F32, tag="yout")
            for j2 in range(max(1, c.D // 512)):
                W = min(512, c.D)
                psY = ps_g2.tile([P, W], F32, tag="g2")
                for hc in range(c.HCH):
                    nc.tensor.matmul(psY[:], lhsT=mT[:, hc, :],
                                     rhs=w2t[:, hc, j2 * W:(j2 + 1) * W],
                                     start=(hc == 0), stop=(hc == c.HCH - 1))
                nc.vector.tensor_scalar(yout[:, j2 * W:(j2 + 1) * W], psY[:], fs[:], None,
                                        op0=OP.mult)
            nc.sync.dma_start(out=rows_out[T * P:(T + 1) * P, :], in_=yout[:])


def shard_inputs(cfg, x, w_gate, w_in, w_out):
    c = cfg
    xf = np.ascontiguousarray(np.asarray(x, dtype=np.float32).reshape(-1, c.D))
    xpad = np.concatenate([xf, np.zeros((1, c.D), np.float32)], axis=0)
    wg = np.ascontiguousarray(np.asarray(w_gate, dtype=np.float32))
    in_maps = []
    for e in range(c.CORES):
        in_maps.append({
            "xpad": xpad,
            "xsh": np.ascontiguousarray(xf[e * c.SH:(e + 1) * c.SH]),
            "wg": wg,
            "w1": np.ascontiguousarray(np.asarray(w_in[e], dtype=np.float32)),
            "w2": np.ascontiguousarray(np.asarray(w_out[e], dtype=np.float32)),
            "eidrows": (np.arange(8, dtype=np.int32) * 8 + e)[:, None],
        })
    return in_maps


def combine_outputs(cfg, results, bias, batch_shape):
    c = cfg
    y_ext = np.zeros((c.NTOK + 1, c.D), dtype=np.float32)
    for e in range(c.CORES):
        r = results[e]
        tok = r["lg_out"][:c.CAP, 0].astype(np.int64)
        np.clip(tok, 0, c.NTOK, out=tok)
        y_ext[tok] += r["rows_out"]
    y = y_ext[:c.NTOK] + np.asarray(bias, np.float32)[None, :]
    psum = np.zeros(8, np.float64)
    fsum = np.zeros(8, np.float64)
    zsum = 0.0
    for e in range(c.CORES):
        lp = results[e]["lossp"][0]
        psum += lp[0:8]
        fsum += lp[8:16]
        zsum += lp[16]
    switchloss = c.E * np.sum((psum / psum.sum()) * (fsum / fsum.sum()))
    zloss = zsum / c.NTOK
    loss = np.float32(switchloss + 0.1 * zloss)
    return y.reshape(*batch_shape, c.D), loss


_CACHE = {}


def _get_kernel(cfg):
    key = (cfg.D, cfg.H, cfg.NTOK, cfg.CAP, cfg.CORES)
    if key not in _CACHE:
        _CACHE[key] = build(cfg)
    return _CACHE[key]


def kernel(x, w_gate, w_in, w_out, bias):
    x = np.asarray(x)
    B, S, D = x.shape
    cfg = Cfg(D=D, H=np.asarray(w_out).shape[2], E=np.asarray(w_in).shape[0],
              NTOK=B * S)
    nc = _get_kernel(cfg)
    in_maps = shard_inputs(cfg, x, np.asarray(w_gate), np.asarray(w_in),
                           np.asarray(w_out))
    res = run_bass_kernel_spmd(nc, in_maps, list(range(cfg.CORES)))
    y, loss = combine_outputs(cfg, res.results, np.asarray(bias), (B, S))
    return y, loss
